# revision 35
# baseline (speedup 1.0000x reference)
"""MlpAttentionLayer Trainium2 kernel (v3 — wide ops, chunked DMA).

Math (reference):
  cat = [x, x-q, q]; h = BN1(cat); p = relu(h @ W1)
  g = BN2(p); w = sigmoid(g @ W2); out = sum_t x * w

Folded:  pre = x @ Wx + Qp[b];  logits = relu(pre) @ W2p + c2.

Device plan (per core, 256 batch rows):
  Host pre-transposes x to bf16 xT [D, BSH, T]; the kernel DMAs it in 4
  chunks of 64 rows (25.6 KB contiguous per partition -> near-line-rate).
  Per group of 4 rows: two paired N=400 matmuls each for pre (Wx
  stationary), the Qp one-hot add, and logits (128-replicated W2p
  stationary, so sigmoid lands broadcast across partitions).  relu and the
  sigmoid run as single wide instructions (DVE/ACT rotated).  The final
  weighted sum is one wide bf16 multiply (DVE/GPSIMD rotated) plus one
  segmented tensor_reduce [128,4,200]->[128,4].  Epilogue transposes
  outT [d,b] -> [b,d].
"""

import sys

sys.path.insert(0, "/opt/trn_rl_repo")

import numpy as np
import ml_dtypes

BN_EPS = 1e-3
B, T, D = 2048, 200, 128
N_CORES = 8
BSH = B // N_CORES          # 256 batch elements per core
G = 4                       # batch elements per pipeline group
NGRP = BSH // G             # 64 groups
PRES = 256                  # per-b PSUM stride (fp32): a b-pair shares a bank
# ramped input-DMA chunk sizes (in groups): small first chunks so the PE
# starts ~2us in instead of waiting for a full 12.8KB/partition transfer
CHUNK_GROUPS = [1, 1, 2, 4, 8, 8, 8, 8, 8, 8, 8]
assert sum(CHUNK_GROUPS) == NGRP
CHUNK_STARTS = [sum(CHUNK_GROUPS[:i]) for i in range(len(CHUNK_GROUPS))]
GPC_MAX = max(CHUNK_GROUPS)

BF16 = ml_dtypes.bfloat16

# relu engine per group (v=DVE, a=ACT) and mult engine (v=DVE, g=GPSIMD)
RELU_PATTERN = "aaavv"
MULT_PATTERN = "gggggggggggggvvv"


def _build_bass():
    from concourse import bacc, mybir
    from concourse.tile import TileContext
    from concourse.masks import make_identity

    fp32 = mybir.dt.float32
    bf16 = mybir.dt.bfloat16
    AF = mybir.ActivationFunctionType
    OP = mybir.AluOpType

    nc = bacc.Bacc()
    xt_d = nc.dram_tensor("xt", (D, BSH, T), bf16, kind="ExternalInput")
    qp_d = nc.dram_tensor("qp", (BSH, D), bf16, kind="ExternalInput")
    wx_d = nc.dram_tensor("wx", (D, D), bf16, kind="ExternalInput")
    w2r_d = nc.dram_tensor("w2r", (D, 128), bf16, kind="ExternalInput")
    c2_d = nc.dram_tensor("c2", (1, 1), fp32, kind="ExternalInput")
    out_d = nc.dram_tensor("out", (BSH, D), fp32, kind="ExternalOutput")

    with TileContext(nc) as tc:
        with (
            tc.tile_pool(name="const", bufs=1) as cpool,
            tc.tile_pool(name="xt", bufs=3) as xtpool,
            tc.tile_pool(name="h1", bufs=6) as h1pool,
            tc.tile_pool(name="sig", bufs=6) as sigpool,
            tc.tile_pool(name="scr", bufs=6) as scrpool,
            tc.tile_pool(name="fin", bufs=1) as finpool,
            tc.tile_pool(name="ps_pre", bufs=4, space="PSUM") as prepool,
        ):
            ident32 = cpool.tile([128, 128], fp32)
            make_identity(nc, ident32)
            ident16 = cpool.tile([128, 128], bf16)
            make_identity(nc, ident16)
            wx_sb = cpool.tile([D, D], bf16)
            nc.sync.dma_start(wx_sb, wx_d[:, :])
            w2r_sb = cpool.tile([D, 128], bf16)
            nc.sync.dma_start(w2r_sb, w2r_d[:, :])
            c2_sb = cpool.tile([128, 1], fp32)
            nc.sync.dma_start(c2_sb, c2_d[0, 0:1].broadcast_to((128, 1)))
            # Qp slabs: [K=128 b-slots, M=128] stationary for the one-hot add
            qp_sb = cpool.tile([128, 2, D], bf16)
            nc.sync.dma_start(
                qp_sb, qp_d[:, :].rearrange("(s k) d -> k s d", k=128)
            )

            outT = finpool.tile([128, BSH], fp32)

            for gi in range(NGRP):
                b0 = gi * G
                # ---- ramped chunked input DMA (contiguous per partition)
                if gi in CHUNK_STARTS:
                    ck = CHUNK_STARTS.index(gi)
                    ng = CHUNK_GROUPS[ck]
                    xt = xtpool.tile([D, GPC_MAX * G, T], bf16, tag="xt")
                    nc.sync.dma_start(
                        xt[:, 0 : ng * G, :],
                        xt_d[:, gi * G : (gi + ng) * G, :],
                    )
                    chunk_base = gi
                go = (gi - chunk_base) * G   # group offset within chunk
                # ---- preT = Wx^T @ xT, paired N=400 matmuls.  A b-pair is
                # packed contiguously (400 fp32) inside one 2KB PSUM bank.
                pre = prepool.tile([128, 2 * 512], fp32, tag="pre")
                pre_p = pre.rearrange("p (h c) -> p h c", c=512)
                for h in range(2):
                    nc.tensor.matmul(
                        pre_p[:, h, 0 : 2 * T],
                        wx_sb,
                        xt[:, go + 2 * h : go + 2 * h + 2, :],
                        start=True, stop=False,
                    )
                # ---- += Qp via one-hot (paired)
                slab = (b0 // 128) % 2
                for h in range(2):
                    k = (b0 + 2 * h) % 128
                    nc.tensor.matmul(
                        pre_p[:, h, 0 : 2 * T].rearrange(
                            "p (g c) -> p g c", c=T
                        ),
                        qp_sb[:, slab, :],
                        ident16[:, k : k + 2]
                        .rearrange("p (g u) -> p g u", u=1)
                        .broadcast_to((128, 2, T)),
                        start=False, stop=True,
                    )
                # ---- h1 = relu(pre), one wide instr (rotated DVE/ACT)
                h1 = h1pool.tile([128, G, T], bf16, tag="h1")
                h1f = h1.rearrange("p g c -> p (g c)")
                pre_w = pre_p[:, :, 0 : 2 * T]
                with tc.high_priority(offset=40):
                    if RELU_PATTERN[gi % len(RELU_PATTERN)] == "a":
                        nc.scalar.activation(h1f, pre_w, AF.Relu)
                    else:
                        nc.vector.tensor_scalar_max(h1f, pre_w, 0.0)
                # ---- logits (128-replicated stationary), paired, into the
                # pre tile's banks (WAR matches the h1 data dependency)
                for h in range(2):
                    nc.tensor.matmul(
                        pre_p[:, h, 0 : 2 * T],
                        w2r_sb,
                        h1f[:, h * 2 * T : (h + 1) * 2 * T],
                        start=True, stop=True,
                    )
                # ---- sigmoid, one wide instr, broadcast layout
                sg = sigpool.tile([128, G, T], bf16, tag="sg")
                nc.scalar.activation(
                    sg.rearrange("p g c -> p (g c)"), pre_w,
                    AF.Sigmoid, bias=c2_sb[:, 0:1],
                )
                # ---- weighted sum: wide mult then segmented reduce
                scr = scrpool.tile([128, G, T], bf16, tag="scr")
                meng = MULT_PATTERN[gi % len(MULT_PATTERN)]
                eng = nc.gpsimd if meng == "g" else nc.vector
                eng.tensor_tensor(
                    scr, xt[:, go : go + G, :], sg, OP.mult
                )
                nc.vector.tensor_reduce(
                    outT[:, b0 : b0 + G], scr, mybir.AxisListType.X, OP.add
                )

            # ---- epilogue: transpose [d, b] -> [b, d] and store
            obt = finpool.tile([128, BSH], fp32)
            for half in range(2):
                ot = prepool.tile([128, 2 * 512], fp32, tag="pre")
                nc.tensor.transpose(
                    ot[:, 0:128], outT[:, half * 128 : half * 128 + 128], ident32
                )
                nc.scalar.activation(
                    obt[:, half * 128 : half * 128 + 128], ot[:, 0:128], AF.Copy
                )
                nc.sync.dma_start(
                    out_d[half * 128 : half * 128 + 128, :],
                    obt[:, half * 128 : half * 128 + 128],
                )
    nc.finalize()
    return nc


_NC_CACHE = {}


def _get_nc():
    if "nc" not in _NC_CACHE:
        _NC_CACHE["nc"] = _build_bass()
    return _NC_CACHE["nc"]


def _prep_host(
    inputs, query, W1, W2,
    bn1_gamma, bn1_beta, bn1_mean, bn1_var,
    bn2_gamma, bn2_beta, bn2_mean, bn2_var,
):
    """Fold BN into weights, precompute Qp, pre-transpose x; returns in_maps."""
    x = np.asarray(inputs, np.float32)
    q = np.asarray(query, np.float64)
    W1 = np.asarray(W1, np.float64)
    W2 = np.asarray(W2, np.float64)
    s1 = np.asarray(bn1_gamma, np.float64) / np.sqrt(
        np.asarray(bn1_var, np.float64) + BN_EPS
    )
    W1s = s1[:, None] * W1
    Wx = W1s[0:D] + W1s[D : 2 * D]               # [D, D]
    Wq = W1s[2 * D : 3 * D] - W1s[D : 2 * D]     # [D, D]
    bias0 = (
        np.asarray(bn1_beta, np.float64) - np.asarray(bn1_mean, np.float64) * s1
    ) @ W1
    Qp = q @ Wq + bias0                          # [B, D]
    s2 = np.asarray(bn2_gamma, np.float64) / np.sqrt(
        np.asarray(bn2_var, np.float64) + BN_EPS
    )
    W2p = s2 * W2[:, 0]                          # [D]
    c2 = float(
        (np.asarray(bn2_beta, np.float64) - np.asarray(bn2_mean, np.float64) * s2)
        @ W2[:, 0]
    )

    wx16 = Wx.astype(BF16)                       # lhsT [K=din, M=dout]
    w2r16 = np.repeat(W2p.astype(BF16)[:, None], 128, axis=1)  # [D, 128]
    qp16 = Qp.astype(BF16)                       # [B, D]
    c2a = np.full((1, 1), c2, np.float32)

    x16 = x.astype(BF16)                         # [B, T, D]
    in_maps = []
    for c in range(N_CORES):
        xs = x16[c * BSH : (c + 1) * BSH]        # [BSH, T, D]
        xtc = np.ascontiguousarray(xs.transpose(2, 0, 1))  # [D, BSH, T]
        in_maps.append(
            {
                "xt": xtc,
                "qp": qp16[c * BSH : (c + 1) * BSH],
                "wx": wx16,
                "w2r": w2r16,
                "c2": c2a,
            }
        )
    return in_maps


def kernel(**inputs):
    from concourse.bass_utils import run_bass_kernel_spmd

    in_maps = _prep_host(**inputs)
    nc = _get_nc()
    res = run_bass_kernel_spmd(nc, in_maps, core_ids=list(range(N_CORES)))
    out = np.concatenate([r["out"] for r in res.results], axis=0)
    return out.astype(np.float32)


# revision 36
# speedup vs baseline: 1.1906x; 1.1906x over previous
"""MlpAttentionLayer Trainium2 kernel (v3 — wide ops, chunked DMA).

Math (reference):
  cat = [x, x-q, q]; h = BN1(cat); p = relu(h @ W1)
  g = BN2(p); w = sigmoid(g @ W2); out = sum_t x * w

Folded:  pre = x @ Wx + Qp[b];  logits = relu(pre) @ W2p + c2.

Device plan (per core, 256 batch rows):
  Host pre-transposes x to bf16 xT [D, BSH, T]; the kernel DMAs it in 4
  chunks of 64 rows (25.6 KB contiguous per partition -> near-line-rate).
  Per group of 4 rows: two paired N=400 matmuls each for pre (Wx
  stationary), the Qp one-hot add, and logits (128-replicated W2p
  stationary, so sigmoid lands broadcast across partitions).  relu and the
  sigmoid run as single wide instructions (DVE/ACT rotated).  The final
  weighted sum is one wide bf16 multiply (DVE/GPSIMD rotated) plus one
  segmented tensor_reduce [128,4,200]->[128,4].  Epilogue transposes
  outT [d,b] -> [b,d].
"""

import sys

sys.path.insert(0, "/opt/trn_rl_repo")

import numpy as np
import ml_dtypes

BN_EPS = 1e-3
B, T, D = 2048, 200, 128
N_CORES = 8
BSH = B // N_CORES          # 256 batch elements per core
G = 4                       # batch elements per pipeline group
NGRP = BSH // G             # 64 groups
PRES = 256                  # per-b PSUM stride (fp32): a b-pair shares a bank
NCHUNK = 8                  # input DMA chunks
GPC = NGRP // NCHUNK        # groups per chunk
BPC = BSH // NCHUNK         # batch rows per chunk

BF16 = ml_dtypes.bfloat16

# relu engine per group (v=DVE, a=ACT) and mult engine (v=DVE, g=GPSIMD)
RELU_PATTERN = "aaavv"
MULT_PATTERN = "gggggggggggggvvv"


def _build_bass():
    from concourse import bacc, mybir
    from concourse.tile import TileContext
    from concourse.masks import make_identity

    fp32 = mybir.dt.float32
    bf16 = mybir.dt.bfloat16
    AF = mybir.ActivationFunctionType
    OP = mybir.AluOpType

    nc = bacc.Bacc()
    xt_d = nc.dram_tensor("xt", (D, BSH, T), bf16, kind="ExternalInput")
    qp_d = nc.dram_tensor("qp", (BSH, D), bf16, kind="ExternalInput")
    wx_d = nc.dram_tensor("wx", (D, D), bf16, kind="ExternalInput")
    w2r_d = nc.dram_tensor("w2r", (D, 128), bf16, kind="ExternalInput")
    c2_d = nc.dram_tensor("c2", (1, 1), fp32, kind="ExternalInput")
    out_d = nc.dram_tensor("out", (BSH, D), fp32, kind="ExternalOutput")

    with TileContext(nc) as tc:
        with (
            tc.tile_pool(name="const", bufs=1) as cpool,
            tc.tile_pool(name="xt", bufs=3) as xtpool,
            tc.tile_pool(name="h1", bufs=6) as h1pool,
            tc.tile_pool(name="sig", bufs=6) as sigpool,
            tc.tile_pool(name="scr", bufs=6) as scrpool,
            tc.tile_pool(name="fin", bufs=1) as finpool,
            tc.tile_pool(name="ps_pre", bufs=4, space="PSUM") as prepool,
        ):
            ident32 = cpool.tile([128, 128], fp32)
            make_identity(nc, ident32)
            ident16 = cpool.tile([128, 128], bf16)
            make_identity(nc, ident16)
            wx_sb = cpool.tile([D, D], bf16)
            nc.sync.dma_start(wx_sb, wx_d[:, :])
            w2r_sb = cpool.tile([D, 128], bf16)
            nc.sync.dma_start(w2r_sb, w2r_d[:, :])
            c2_sb = cpool.tile([128, 1], fp32)
            nc.sync.dma_start(c2_sb, c2_d[0, 0:1].broadcast_to((128, 1)))
            # Qp slabs: [K=128 b-slots, M=128] stationary for the one-hot add
            qp_sb = cpool.tile([128, 2, D], bf16)
            nc.sync.dma_start(
                qp_sb, qp_d[:, :].rearrange("(s k) d -> k s d", k=128)
            )

            outT = finpool.tile([128, BSH], fp32)

            for gi in range(NGRP):
                b0 = gi * G
                # ---- chunked input DMA (25.6 KB contiguous per partition)
                if gi % GPC == 0:
                    ck = gi // GPC
                    xt = xtpool.tile([D, BPC, T], bf16, tag="xt")
                    nc.sync.dma_start(
                        xt, xt_d[:, ck * BPC : (ck + 1) * BPC, :]
                    )
                go = (gi % GPC) * G     # group offset within chunk
                # ---- preT = Wx^T @ xT, paired N=400 matmuls.  A b-pair is
                # packed contiguously (400 fp32) inside one 2KB PSUM bank.
                pre = prepool.tile([128, 2 * 512], fp32, tag="pre")
                pre_p = pre.rearrange("p (h c) -> p h c", c=512)
                for h in range(2):
                    nc.tensor.matmul(
                        pre_p[:, h, 0 : 2 * T],
                        wx_sb,
                        xt[:, go + 2 * h : go + 2 * h + 2, :],
                        start=True, stop=False,
                    )
                # ---- += Qp via one-hot (paired)
                slab = (b0 // 128) % 2
                for h in range(2):
                    k = (b0 + 2 * h) % 128
                    nc.tensor.matmul(
                        pre_p[:, h, 0 : 2 * T].rearrange(
                            "p (g c) -> p g c", c=T
                        ),
                        qp_sb[:, slab, :],
                        ident16[:, k : k + 2]
                        .rearrange("p (g u) -> p g u", u=1)
                        .broadcast_to((128, 2, T)),
                        start=False, stop=True,
                    )
                # ---- h1 = relu(pre), one wide instr (rotated DVE/ACT)
                h1 = h1pool.tile([128, G, T], bf16, tag="h1")
                h1f = h1.rearrange("p g c -> p (g c)")
                pre_w = pre_p[:, :, 0 : 2 * T]
                with tc.high_priority(offset=40):
                    if RELU_PATTERN[gi % len(RELU_PATTERN)] == "a":
                        nc.scalar.activation(h1f, pre_w, AF.Relu)
                    else:
                        nc.vector.tensor_scalar_max(h1f, pre_w, 0.0)
                # ---- logits (128-replicated stationary), paired, into the
                # pre tile's banks (WAR matches the h1 data dependency)
                for h in range(2):
                    nc.tensor.matmul(
                        pre_p[:, h, 0 : 2 * T],
                        w2r_sb,
                        h1f[:, h * 2 * T : (h + 1) * 2 * T],
                        start=True, stop=True,
                    )
                # ---- sigmoid, one wide instr, broadcast layout
                sg = sigpool.tile([128, G, T], bf16, tag="sg")
                nc.scalar.activation(
                    sg.rearrange("p g c -> p (g c)"), pre_w,
                    AF.Sigmoid, bias=c2_sb[:, 0:1],
                )
                # ---- weighted sum: wide mult then segmented reduce
                scr = scrpool.tile([128, G, T], bf16, tag="scr")
                meng = MULT_PATTERN[gi % len(MULT_PATTERN)]
                eng = nc.gpsimd if meng == "g" else nc.vector
                eng.tensor_tensor(
                    scr, xt[:, go : go + G, :], sg, OP.mult
                )
                nc.vector.tensor_reduce(
                    outT[:, b0 : b0 + G], scr, mybir.AxisListType.X, OP.add
                )

            # ---- epilogue: transpose [d, b] -> [b, d] and store
            obt = finpool.tile([128, BSH], fp32)
            for half in range(2):
                ot = prepool.tile([128, 2 * 512], fp32, tag="pre")
                nc.tensor.transpose(
                    ot[:, 0:128], outT[:, half * 128 : half * 128 + 128], ident32
                )
                nc.scalar.activation(
                    obt[:, half * 128 : half * 128 + 128], ot[:, 0:128], AF.Copy
                )
                nc.sync.dma_start(
                    out_d[half * 128 : half * 128 + 128, :],
                    obt[:, half * 128 : half * 128 + 128],
                )
    nc.finalize()
    return nc


_NC_CACHE = {}


def _get_nc():
    if "nc" not in _NC_CACHE:
        _NC_CACHE["nc"] = _build_bass()
    return _NC_CACHE["nc"]


def _prep_host(
    inputs, query, W1, W2,
    bn1_gamma, bn1_beta, bn1_mean, bn1_var,
    bn2_gamma, bn2_beta, bn2_mean, bn2_var,
):
    """Fold BN into weights, precompute Qp, pre-transpose x; returns in_maps."""
    x = np.asarray(inputs, np.float32)
    q = np.asarray(query, np.float64)
    W1 = np.asarray(W1, np.float64)
    W2 = np.asarray(W2, np.float64)
    s1 = np.asarray(bn1_gamma, np.float64) / np.sqrt(
        np.asarray(bn1_var, np.float64) + BN_EPS
    )
    W1s = s1[:, None] * W1
    Wx = W1s[0:D] + W1s[D : 2 * D]               # [D, D]
    Wq = W1s[2 * D : 3 * D] - W1s[D : 2 * D]     # [D, D]
    bias0 = (
        np.asarray(bn1_beta, np.float64) - np.asarray(bn1_mean, np.float64) * s1
    ) @ W1
    Qp = q @ Wq + bias0                          # [B, D]
    s2 = np.asarray(bn2_gamma, np.float64) / np.sqrt(
        np.asarray(bn2_var, np.float64) + BN_EPS
    )
    W2p = s2 * W2[:, 0]                          # [D]
    c2 = float(
        (np.asarray(bn2_beta, np.float64) - np.asarray(bn2_mean, np.float64) * s2)
        @ W2[:, 0]
    )

    wx16 = Wx.astype(BF16)                       # lhsT [K=din, M=dout]
    w2r16 = np.repeat(W2p.astype(BF16)[:, None], 128, axis=1)  # [D, 128]
    qp16 = Qp.astype(BF16)                       # [B, D]
    c2a = np.full((1, 1), c2, np.float32)

    x16 = x.astype(BF16)                         # [B, T, D]
    in_maps = []
    for c in range(N_CORES):
        xs = x16[c * BSH : (c + 1) * BSH]        # [BSH, T, D]
        xtc = np.ascontiguousarray(xs.transpose(2, 0, 1))  # [D, BSH, T]
        in_maps.append(
            {
                "xt": xtc,
                "qp": qp16[c * BSH : (c + 1) * BSH],
                "wx": wx16,
                "w2r": w2r16,
                "c2": c2a,
            }
        )
    return in_maps


def kernel(**inputs):
    from concourse.bass_utils import run_bass_kernel_spmd

    in_maps = _prep_host(**inputs)
    nc = _get_nc()
    res = run_bass_kernel_spmd(nc, in_maps, core_ids=list(range(N_CORES)))
    out = np.concatenate([r["out"] for r in res.results], axis=0)
    return out.astype(np.float32)
